# revision 1
# baseline (speedup 1.0000x reference)
"""Trainium2 Bass kernel for the BDH-style recurrent block.

Strategy: data-parallel over B (8 batches -> 8 NeuronCores, no collectives).
The T=128-step scan is de-sequentialized into dense matmuls per core:

  u_t = relu(emb_t @ Dx.T)                                  (T,N) batched matmul
  x_t = (XD*x_{t-1} + u_t)/s_t  with s_t = XD + sum(u_t)    (L1 norm; x>=0)
      => x = C @ u, C[t,s] = (1/s_s) exp(A_t - A_s), A_t = cumsum log(XD/s_r)
  a*_t = rho_{t-1} @ x_t = ((DecayMask . X X^T) @ ln(emb))_t   (rho_0 = 0)
  y_t  = relu(ln(a*_t) @ Dy.T) * x_t                        (x_t >= 0)
  v*_t = ln(y_t @ E.T)

Matmuls run in float32r (~1.5e-4 rounding, 4x faster PE streaming at free
dims >= 256). X/Y live in (t, n)-major layout; PE transposes provide the
n-major tiles needed for the Gram matrix and the E contraction. A bf16
dummy-matmul warmup during the initial weight DMA lifts the PE HAM clock
gate to 2.4 GHz before the real work arrives.
"""

import math
from contextlib import ExitStack

import numpy as np

N = 2048
D = 256
B = 8
T = 128
XD = 0.97
UD = 0.97
LN_EPS = 1e-5
L1_EPS = 1e-12

# log-domain recentring: E[sum relu(N(0,1)) over 2048] + XD ~ 818.9
LNC2INV = 6.7065
C2 = math.exp(-LNC2INV)
K1 = LNC2INV - math.log(XD)

KD = D // 128   # 2
KN = N // 128   # 16
NJ = N // 512   # 4
WARMUP_MMS = 8   # 512-col bf16 MMs, ~0.53us each cold: ~4.3us dense warmup
NCONST = 5 * T + 1  # packed const block columns


def _pack_jk(wT):
    # (KD,128,N) k-major -> (128, [j(4), k(2), 512]) per-partition contiguous
    return np.ascontiguousarray(
        wT.reshape(KD, 128, NJ, 512).transpose(1, 2, 0, 3).reshape(128, KD * N))

_cache = {}
SIM_MODE = False  # CoreSim's xorwow random-fill is broken; use memset there


def _consts():
    """Packed (128, NCONST) const block: [utones | trik | dmaskT | ident |
    negblock | xdvec-col]. One contiguous DMA."""
    r = np.arange(T)
    utones = (r[:, None] <= r[None, :]).astype(np.float32)          # [r,t] r<=t
    tri = r[None, :] - r[:, None]                                   # t - s
    trik = np.where(tri >= 0, -K1 * tri - LNC2INV, -10000.0).astype(np.float32)
    pw = r[:, None] - 1 - r[None, :]                                # [t,s] t-1-s
    dmask = np.where(pw >= 0, UD ** np.maximum(pw, 0), 0.0).astype(np.float32)
    dmaskT = np.ascontiguousarray(dmask.T)                          # [s,t]
    ident = np.eye(T, dtype=np.float32)
    negblock = -np.ones((T, T), dtype=np.float32)
    xdvec = np.full((T, 1), C2 * XD, dtype=np.float32)
    xdvec[0, 0] = 0.0                                               # x_{-1} = 0
    return np.ascontiguousarray(np.concatenate(
        [utones, trik, dmaskT, ident, negblock, xdvec], axis=1))


def _split_multiwait(nc, mybir):
    """This walrus build caps sync waits per instruction (1 for regular
    instructions, 2 for EventSemaphore). Tile attaches more (e.g. the
    kernel-tail Drain waits on every live semaphore). Hoist excess waits
    onto same-engine NOPs placed immediately before the instruction —
    engine queues are sequential, so semantics are preserved."""
    n = 0
    for f in nc.m.functions:
        for bb in f.blocks:
            out = []
            changed = False
            for ins in bb.instructions:
                si = ins.sync_info
                ow = list(si.on_wait) if si is not None else []
                cap = 2 if ins.opcode == "EventSemaphore" else 1
                if len(ow) > cap:
                    sem_waits = [w for w in ow if w.sync_type == "semaphore"]
                    other = [w for w in ow if w.sync_type != "semaphore"]
                    keep = max(cap - len(other), 0)
                    hoist = sem_waits[:len(sem_waits) - keep] if keep else sem_waits
                    kept = sem_waits[len(hoist):] + other
                    assert len(kept) <= cap, (len(kept), cap, ins.opcode)
                    changed = True
                    for w in hoist:
                        n += 1
                        nop = mybir.InstNoOp(
                            name=f"wsplit-{n}",
                            sync_info=mybir.SyncInfo(on_wait=[w], on_update=[]),
                            bass_nofuse=True,
                            engine=ins.engine,
                        )
                        nc.register_instruction(nop, overwrite=True)
                        out.append(nop)
                    si.on_wait = kept
                out.append(ins)
            if changed:
                bb.instructions = out
    return nc


def _build():
    import concourse.bass as bass
    import concourse.mybir as mybir
    import concourse.tile as tile

    f32 = mybir.dt.float32
    f32r = mybir.dt.float32r
    bf16 = mybir.dt.bfloat16
    AF = mybir.ActivationFunctionType
    ALU = mybir.AluOpType
    AX = mybir.AxisListType

    from concourse.vector_clock import ScopedClock

    class _TrimTailTC(tile.TileContext):
        # Drop the second kernel-tail all-engine barrier: it only orders
        # the semaphore resets against engine halt, and nothing executes
        # after it. The first barrier (before resets) is kept, so resets
        # still happen on a quiesced machine and re-execution stays safe.
        def _drain_and_barrier(self, tick_clock, wait_clock):
            drain_inst = self.nc.sync.drain()
            wait_clock.add_sem_waits(
                drain_inst.ins, ScopedClock({None: tick_clock.global_clock})
            )
            self.nc.all_engine_barrier()
            assert self.sems is not None
            popped = self.nc._tile_sem_poison_stack.pop()
            assert popped is self._sem_poison
            self.nc.clear_and_free_semaphores(
                list(self.sems.allocated().values())
            )

    nc = bass.Bass()

    d_emb = nc.dram_tensor("emb", [T, D], f32, kind="ExternalInput")
    d_embT = nc.dram_tensor("embT", [128, KD * T], f32, kind="ExternalInput")
    d_dxT = nc.dram_tensor("dxT", [128, KD * N], f32, kind="ExternalInput")
    d_dyT = nc.dram_tensor("dyT", [128, KD * N], f32, kind="ExternalInput")
    d_eT = nc.dram_tensor("eT", [128, KN * D], f32, kind="ExternalInput")
    d_consts = nc.dram_tensor("consts", [128, NCONST], f32, kind="ExternalInput")
    d_out = nc.dram_tensor("out", [T, D], f32, kind="ExternalOutput")

    with _TrimTailTC(nc) as tc, ExitStack() as ctx:
        work = ctx.enter_context(tc.tile_pool(name="work", bufs=1))
        stats = ctx.enter_context(tc.tile_pool(name="stats", bufs=1))
        p_u = ctx.enter_context(tc.tile_pool(name="p_u", bufs=2, space="PSUM"))
        p_sq = ctx.enter_context(tc.tile_pool(name="p_sq", bufs=4, space="PSUM"))
        p_g = ctx.enter_context(tc.tile_pool(name="p_g", bufs=1, space="PSUM"))
        p_med = ctx.enter_context(tc.tile_pool(name="p_med", bufs=1, space="PSUM"))

        # ---- PE warmup: random-data bf16 matmuls while weights stream ---
        # (all-zero operands leave the HAM activity monitor cold: no
        # switching activity -> the clock gate never lifts to 2.4 GHz)
        wu_sb = work.tile([128, 512], bf16)
        if SIM_MODE:
            nc.vector.memset(wu_sb[:], 1.0)
        else:
            nc.vector.random(wu_sb[:])
        wu_ps = p_u.tile([128, 512], f32, tag="pu")
        for i in range(WARMUP_MMS):
            nc.tensor.matmul(wu_ps[:], wu_sb[:, 0:128], wu_sb[:], start=True,
                             stop=True)

        # ---- activation table preloads (Ln/Exp used mid-kernel) ---------
        pre_sb = stats.tile([1, 1], f32)
        nc.vector.memset(pre_sb[:], 1.0)
        pre_o = stats.tile([1, 1], f32)
        nc.scalar.activation(pre_o[:], pre_sb[:], AF.Ln)
        nc.scalar.activation(pre_o[:], pre_sb[:], AF.Exp)
        nc.scalar.activation(pre_o[:], pre_sb[:], AF.Square)

        # ---- DMAs: qSP-HWDGE executes these in FIFO order, each striped
        # across the 16 SDMA engines at ~full HBM rate. Order and piece
        # granularity = the compute pipeline's start schedule.
        # dxT/dyT are packed [j(4), k(2), 512] so consumer j needs only
        # piece j; eT is chunk-major so vraw group g needs piece g.
        embT_sb = work.tile([128, KD * T], f32r)
        nc.sync.dma_start(embT_sb[:], d_embT[:].bitcast(f32r))
        dxT_sb = work.tile([128, KD * N], f32r)
        for j in range(NJ):
            nc.sync.dma_start(dxT_sb[:, j * 1024:(j + 1) * 1024],
                              d_dxT[:, j * 1024:(j + 1) * 1024].bitcast(f32r))
        consts_sb = work.tile([128, NCONST], f32)
        nc.sync.dma_start(consts_sb[:], d_consts[:])
        utones_sb = consts_sb[:, 0:T]
        trik_sb = consts_sb[:, T:2 * T]
        dmaskT_sb = consts_sb[:, 2 * T:3 * T]
        negones_sb = consts_sb[0:1, 4 * T:5 * T]
        xdvec_sb = consts_sb[:, 5 * T:5 * T + 1]
        emb_sb = work.tile([T, D], f32)
        nc.sync.dma_start(emb_sb[:], d_emb[:])
        ident_t = work.tile([T, T], f32r)
        nc.sync.dma_start(ident_t[:], d_consts[:, 3 * T:4 * T].bitcast(f32r))
        ident_sb = ident_t[:]
        dyT_sb = work.tile([128, KD * N], f32r)
        for j in range(NJ):
            nc.sync.dma_start(dyT_sb[:, j * 1024:(j + 1) * 1024],
                              d_dyT[:, j * 1024:(j + 1) * 1024].bitcast(f32r))
        eT_sb = work.tile([128, KN * D], f32r)
        for g in range(4):
            nc.sync.dma_start(eT_sb[:, g * 1024:(g + 1) * 1024],
                              d_eT[:, g * 1024:(g + 1) * 1024].bitcast(f32r))

        def keepalive(ap):
            # PE matmuls gated on a late LN stat: hold the HAM clock warm
            # (with real switching activity) through serial non-PE stretches.
            ka = p_sq.tile([T, T], f32, tag="sq")
            nc.tensor.matmul(ka[0:1, :], ap, trik_sb[:], start=True, stop=True)
            nc.tensor.matmul(ka[0:1, :], ap, dmaskT_sb[:], start=True, stop=True)

        def fast_ln(psum_src, dst, tagp, hold_pe=False):
            """dst = LN(psum_src) straight out of PSUM: bn_stats/bn_aggr for
            mean+var (one DVE pass), short mostly-DVE scalar chain, one ACT
            pass for the normalize. No SBUF evac needed."""
            stat6 = stats.tile([T, 6], f32, tag=f"{tagp}_s6")
            nc.vector.bn_stats(stat6[:], psum_src)
            mv = stats.tile([T, 2], f32, tag=f"{tagp}_mv")
            nc.vector.bn_aggr(mv[:], stat6[:])
            if hold_pe:
                keepalive(mv[:, 0:1])
            veps = stats.tile([T, 1], f32, tag=f"{tagp}_ve")
            nc.vector.tensor_scalar_add(veps[:], mv[:, 1:2], LN_EPS)
            rv = stats.tile([T, 1], f32, tag=f"{tagp}_rv")
            nc.vector.reciprocal(rv[:], veps[:])
            rstd = stats.tile([T, 1], f32, tag=f"{tagp}_rs")
            nc.scalar.sqrt(rstd[:], rv[:])
            if hold_pe:
                keepalive(rstd[:])
            nmr = stats.tile([T, 1], f32, tag=f"{tagp}_nr")
            nc.vector.scalar_tensor_tensor(nmr[:], mv[:, 0:1], -1.0, rstd[:],
                                           op0=ALU.mult, op1=ALU.mult)
            nc.scalar.activation(dst[:], psum_src, AF.Identity,
                                 scale=rstd[:], bias=nmr[:])

        # ---- all-ACT layernorm helper -----------------------------------
        def layernorm(src, dst, tagp, evac=None, hold_pe=False):
            """dst = LN(src) over free dim. If evac is a PSUM AP, src is
            filled from it (evac+rowsum fused); else src must be SBUF and
            a junk copy produces the rowsum."""
            junk = work.tile([T, D], f32, tag="lnjunk")
            msum = stats.tile([T, 1], f32, tag=f"{tagp}_ms")
            if evac is not None:
                nc.scalar.activation(src[:], evac, AF.Copy, accum_out=msum[:])
            else:
                nc.scalar.activation(junk[:], src[:], AF.Copy, accum_out=msum[:])
            negm = stats.tile([T, 1], f32, tag=f"{tagp}_nm")
            nc.scalar.mul(negm[:], msum[:], -1.0 / D)
            if hold_pe:
                keepalive(negm[:])
            ssum = stats.tile([T, 1], f32, tag=f"{tagp}_ss")
            nc.scalar.activation(junk[:], src[:], AF.Square, bias=negm[:],
                                 accum_out=ssum[:])
            veps = stats.tile([T, 1], f32, tag=f"{tagp}_ve")
            nc.vector.tensor_scalar(veps[:], ssum[:], 1.0 / D, LN_EPS,
                                    op0=ALU.mult, op1=ALU.add)
            lv = stats.tile([T, 1], f32, tag=f"{tagp}_lv")
            nc.scalar.activation(lv[:], veps[:], AF.Ln)
            rstd = stats.tile([T, 1], f32, tag=f"{tagp}_rs")
            nc.scalar.activation(rstd[:], lv[:], AF.Exp, scale=-0.5)
            if hold_pe:
                keepalive(rstd[:])
            nmr = stats.tile([T, 1], f32, tag=f"{tagp}_nr")
            nc.scalar.mul(nmr[:], negm[:], rstd[:])
            nc.scalar.activation(dst[:], src[:], AF.Identity,
                                 scale=rstd[:], bias=nmr[:])

        # ---- u = relu(emb @ Dx.T) (f32r), row sums ----------------------
        u_sb = work.tile([T, N], f32r)
        su_part = stats.tile([T, NJ], f32)
        for j in range(NJ):
            ps = p_u.tile([128, 512], f32, tag="pu")
            for c in range(KD):
                nc.tensor.matmul(
                    ps[:],
                    embT_sb[:, c * T:(c + 1) * T],
                    dxT_sb[:, j * 1024 + c * 512: j * 1024 + (c + 1) * 512],
                    start=(c == 0),
                    stop=(c == KD - 1),
                )
            nc.scalar.activation(
                u_sb[:, j * 512:(j + 1) * 512], ps[:], AF.Relu,
                accum_out=su_part[:, j:j + 1],
            )

        # ---- C^T coefficient matrix -------------------------------------
        su = stats.tile([T, 1], f32)
        nc.vector.tensor_reduce(su[:], su_part[:], axis=AX.X, op=ALU.add)
        keepalive(su[:])
        q_sb = stats.tile([T, 1], f32)
        nc.scalar.activation(q_sb[:], su[:], AF.Ln, scale=C2, bias=xdvec_sb[:])

        qc = p_sq.tile([T, T], f32, tag="sq")               # Q_s column
        nc.tensor.matmul(qc[:, 0:1], utones_sb[:], q_sb[:], start=True, stop=True)
        qr = p_sq.tile([T, T], f32, tag="sq")               # Q_t row
        nc.tensor.matmul(qr[0:1, :], q_sb[:], utones_sb[:], start=True, stop=True)
        qr_sb = stats.tile([1, T], f32)
        nc.vector.tensor_copy(qr_sb[:], qr[0:1, :])
        colsc = stats.tile([T, 1], f32)                     # Q_s - q_s
        nc.vector.tensor_sub(colsc[:], qc[:, 0:1], q_sb[:])
        keepalive(colsc[:])
        bc = p_sq.tile([T, T], f32, tag="sq")               # [s,t] = -Q_t
        nc.tensor.matmul(bc[:], negones_sb[:], qr_sb[:], start=True, stop=True)

        expo = work.tile([T, T], f32)
        nc.vector.scalar_tensor_tensor(
            expo[:], bc[:], colsc[:], trik_sb[:], op0=ALU.add, op1=ALU.add
        )
        expoc = work.tile([T, T], f32)
        nc.vector.tensor_scalar_max(expoc[:], expo[:], -80.0)
        ct_sb = work.tile([T, T], f32r)                     # C^T [s,t]
        nc.scalar.activation(ct_sb[:], expoc[:], AF.Exp)

        # ---- vn = LN(emb) (off critical path) ---------------------------
        vn_sb = work.tile([T, D], f32r)
        fast_ln(emb_sb[:], vn_sb, "vn")

        # ---- X = C @ u (t,n-major, f32r), X^T via PE transpose ----------
        x_sb = work.tile([T, N], f32r)
        for j in range(NJ):
            ps = p_u.tile([128, 512], f32, tag="pu")
            nc.tensor.matmul(ps[:], ct_sb[:], u_sb[:, j * 512:(j + 1) * 512],
                             start=True, stop=True)
            if j % 2 == 0:
                nc.vector.tensor_copy(x_sb[:, j * 512:(j + 1) * 512], ps[:])
            else:
                nc.scalar.copy(x_sb[:, j * 512:(j + 1) * 512], ps[:])

        # ---- X^T via PE transpose, G = X X^T, interleaved per chunk -----
        xt_sb = work.tile([128, N], f32r)
        g = p_g.tile([T, T], f32, tag="g")
        for gq in range(4):
            for cc in range(4):
                c = 4 * gq + cc
                tp = p_sq.tile([T, T], f32, tag="sq")
                nc.tensor.transpose(tp[:].bitcast(f32r),
                                    x_sb[:, c * T:(c + 1) * T], ident_sb)
                if c % 2 == 0:
                    nc.vector.tensor_copy(xt_sb[:, c * T:(c + 1) * T], tp[:])
                else:
                    nc.scalar.copy(xt_sb[:, c * T:(c + 1) * T], tp[:])
            for cc in range(4):
                c = 4 * gq + cc
                nc.tensor.matmul(g[:], xt_sb[:, c * T:(c + 1) * T],
                                 xt_sb[:, c * T:(c + 1) * T],
                                 start=(c == 0), stop=(c == KN - 1))
        wt_sb = work.tile([T, T], f32r)
        nc.vector.tensor_mul(wt_sb[:], g[:], dmaskT_sb[:])

        # ---- a* = W @ vn, LN, transpose ---------------------------------
        aps = p_med.tile([T, D], f32, tag="med")
        nc.tensor.matmul(aps[:], wt_sb[:], vn_sb[:], start=True, stop=True)
        lna_sb = work.tile([T, D], f32r)
        fast_ln(aps[:], lna_sb, "la", hold_pe=True)

        lnaT_sb = work.tile([128, KD * T], f32r)
        for c in range(KD):
            tp = p_sq.tile([T, T], f32, tag="sq")
            nc.tensor.transpose(tp[:].bitcast(f32r), lna_sb[:, c * T:(c + 1) * T],
                                ident_sb)
            nc.scalar.copy(lnaT_sb[:, c * T:(c + 1) * T], tp[:])

        # ---- Ycore (t,n-major, f32r), Y = relu(Ycore) * X ---------------
        # ---- Ycore -> Y -> Y^T -> v_raw, interleaved per j-group --------
        y_sb = work.tile([T, N], f32r)
        yt_sb = work.tile([128, N], f32r)
        vps = p_med.tile([T, D], f32, tag="med")
        for j in range(NJ):
            ps = p_u.tile([128, 512], f32, tag="pu")
            for k in range(KD):
                nc.tensor.matmul(ps[:], lnaT_sb[:, k * T:(k + 1) * T],
                                 dyT_sb[:, j * 1024 + k * 512: j * 1024 + (k + 1) * 512],
                                 start=(k == 0), stop=(k == KD - 1))
            nc.vector.scalar_tensor_tensor(
                y_sb[:, j * 512:(j + 1) * 512], ps[:], 0.0,
                x_sb[:, j * 512:(j + 1) * 512], op0=ALU.max, op1=ALU.mult,
            )
            # transposes+evacs first, then the vraw MMs: the PE executes its
            # queue in order, so this hides each evac under later transposes
            for cc in range(4):
                c = 4 * j + cc
                tp = p_sq.tile([T, T], f32, tag="sq")
                nc.tensor.transpose(tp[:].bitcast(f32r),
                                    y_sb[:, c * T:(c + 1) * T], ident_sb)
                if c % 2 == 0:
                    nc.vector.tensor_copy(yt_sb[:, c * T:(c + 1) * T], tp[:])
                else:
                    nc.scalar.copy(yt_sb[:, c * T:(c + 1) * T], tp[:])
            for cc in range(4):
                c = 4 * j + cc
                nc.tensor.matmul(vps[:], yt_sb[:, c * T:(c + 1) * T],
                                 eT_sb[:, c * D:(c + 1) * D],
                                 start=(c == 0), stop=(c == KN - 1))
        vstar_sb = work.tile([T, D], f32)
        fast_ln(vps[:], vstar_sb, "vs")

        nc.sync.dma_start(d_out[:], vstar_sb[:])

    return _split_multiwait(nc, mybir)


def _numpy_fallback(embeddings, E, Dx, Dy, x_state, rho_state):
    # General-path reference (only used if initial states are nonzero).
    def ln(x):
        m = x.mean(-1, keepdims=True)
        v = ((x - m) ** 2).mean(-1, keepdims=True)
        return (x - m) / np.sqrt(v + LN_EPS)

    x_s = x_state.astype(np.float32).copy()
    rho = rho_state.astype(np.float32).copy()
    outs = np.zeros((B, T, D), dtype=np.float32)
    for t in range(T):
        v_prev = embeddings[:, t, :]
        x_upd = np.maximum(v_prev @ Dx.T, 0.0)
        x_t = XD * x_s + x_upd
        x_t = x_t / np.maximum(np.abs(x_t).sum(-1, keepdims=True), L1_EPS)
        a_star = np.einsum("bdn,bn->bd", rho, x_t)
        y_core = ln(a_star) @ Dy.T
        y_t = np.maximum(y_core, 0.0) * np.maximum(x_t, 0.0)
        outs[:, t, :] = ln(y_t @ E.T)
        vn = ln(v_prev)
        rho = UD * rho + np.einsum("bd,bn->bdn", vn, x_t)
        x_s = x_t
    return outs


def kernel(embeddings, E, Dx, Dy, x_state, rho_state):
    embeddings = np.ascontiguousarray(embeddings, dtype=np.float32)
    E = np.ascontiguousarray(E, dtype=np.float32)
    Dx = np.ascontiguousarray(Dx, dtype=np.float32)
    Dy = np.ascontiguousarray(Dy, dtype=np.float32)

    if np.any(x_state) or np.any(rho_state):
        return _numpy_fallback(embeddings, E, Dx, Dy,
                               np.asarray(x_state, np.float32),
                               np.asarray(rho_state, np.float32))

    from concourse.bass_utils import run_bass_kernel_spmd

    if "nc" not in _cache:
        _cache["nc"] = _build()
    nc = _cache["nc"]

    consts = _consts()
    # SBUF-layout packing: row p holds that partition's contiguous span.
    dxT = _pack_jk(Dx.T.reshape(KD, 128, N))
    dyT = _pack_jk(Dy.T.reshape(KD, 128, N))
    eT = np.ascontiguousarray(
        E.T.reshape(KN, 128, D).transpose(1, 0, 2).reshape(128, KN * D))

    in_maps = []
    for b in range(B):
        emb_b = embeddings[b]
        embT_b = np.ascontiguousarray(
            emb_b.T.reshape(KD, 128, T).transpose(1, 0, 2).reshape(128, KD * T))
        in_maps.append({
            "emb": emb_b,
            "embT": embT_b,
            "dxT": dxT,
            "dyT": dyT,
            "eT": eT,
            "consts": consts,
        })

    res = run_bass_kernel_spmd(nc, in_maps, list(range(B)))
    _cache["last_results"] = res
    return np.stack([res.results[i]["out"] for i in range(B)])



# revision 11
# speedup vs baseline: 1.0860x; 1.0860x over previous
"""Trainium2 Bass kernel for the BDH-style recurrent block.

Strategy: data-parallel over B (8 batches -> 8 NeuronCores, no collectives).
The T=128-step scan is de-sequentialized into dense matmuls per core:

  u_t = relu(emb_t @ Dx.T)                                  (T,N)
  x_t = (XD*x_{t-1} + u_t)/s_t  with s_t = XD + sum(u_t)    (L1 norm; x>=0)
      => x = C @ u, C[t,s] = (1/s_s) exp(A_t - A_s), A_t = cumsum log(XD/s_r)
  a*_t = rho_{t-1} @ x_t = ((DecayMask . X X^T) @ ln(emb))_t   (rho_0 = 0)
  y_t  = relu(ln(a*_t) @ Dy.T) * x_t                        (x_t >= 0)
  v*_t = ln(y_t @ E.T)

All matmul operands are bf16 (f32 PSUM accumulation): halves the weight DMA
(the dominant cost; HBM streaming runs at ~90% of the 360 GB/s roofline so
bytes are the only lever) and streams 1 col/cycle at any free dim (f32r is
4x slower below 256). The dataflow is transpose-free: X^T and Ycore^T are
produced directly in n-major layout by swapping the stationary operand, so
the Gram matrix, the y*x gate, and the E/Dy contractions never round-trip
through PE transposes. a* accumulates per 512-column Gram super-chunk,
removing the full-Gram barrier. Every ACT-engine function is chosen from
the one 'natural_log_exp_and_others' table (rsqrt = exp(-.5 ln)), so the
1.3us table reload is paid once at boot. Dummy bf16 matmuls pad PE idle
gaps to hold the DVFS ramp (full clock needs ~3us of continuous PE busy).
"""

import math
from contextlib import ExitStack

import numpy as np
import ml_dtypes

BF16 = ml_dtypes.bfloat16

N = 2048
D = 256
B = 8
T = 128
XD = 0.97
UD = 0.97
LN_EPS = 1e-5
L1_EPS = 1e-12

# log-domain recentring: E[sum relu(N(0,1)) over 2048] + XD ~ 818.9
LNC2INV = 6.7065
C2 = math.exp(-LNC2INV)
K1 = LNC2INV - math.log(XD)

KD = D // 128   # 2
KN = N // 128   # 16
NJ = N // 512   # 4

NCF = 2 * T + 1   # f32 consts: trik | ident | xdvec
NCB = 3 * T       # bf16 consts: utones | utones_strict | dmaskT

_cache = {}
SIM_MODE = False  # CoreSim's xorwow random-fill is broken; use memset there


def _consts_f():
    r = np.arange(T)
    tri = r[None, :] - r[:, None]                                   # [s,t] t-s
    trik = np.where(tri >= 0, -K1 * tri - LNC2INV, -10000.0).astype(np.float32)
    ident = np.eye(T, dtype=np.float32)
    xdvec = np.full((T, 1), C2 * XD, dtype=np.float32)
    xdvec[0, 0] = 0.0                                               # x_{-1} = 0
    return np.ascontiguousarray(np.concatenate([trik, ident, xdvec], axis=1))


def _consts_b():
    r = np.arange(T)
    utones = (r[:, None] <= r[None, :]).astype(BF16)                # [r,t] r<=t
    ustrict = (r[:, None] < r[None, :]).astype(BF16)                # [r,t] r<t
    pw = r[:, None] - 1 - r[None, :]                                # [t,s] t-1-s
    dmask = np.where(pw >= 0, UD ** np.maximum(pw, 0), 0.0)
    dmaskT = np.ascontiguousarray(dmask.T).astype(BF16)             # [s,t]
    return np.ascontiguousarray(np.concatenate([utones, ustrict, dmaskT],
                                               axis=1))


def _pack_jk(wT):
    # (KD,128,N) k-major -> (128, [j(4), k(2), 512]) per-partition contiguous
    return np.ascontiguousarray(
        wT.reshape(KD, 128, NJ, 512).transpose(1, 2, 0, 3).reshape(128, KD * N))


def _split_multiwait(nc, mybir):
    """This walrus build caps sync waits per instruction (1 for regular
    instructions, 2 for EventSemaphore). Tile attaches more (e.g. the
    kernel-tail Drain waits on every live semaphore). Hoist excess waits
    onto same-engine NOPs placed immediately before the instruction —
    engine queues are sequential, so semantics are preserved."""
    n = 0
    for f in nc.m.functions:
        for bb in f.blocks:
            out = []
            changed = False
            for ins in bb.instructions:
                si = ins.sync_info
                ow = list(si.on_wait) if si is not None else []
                cap = 2 if ins.opcode == "EventSemaphore" else 1
                if len(ow) > cap:
                    sem_waits = [w for w in ow if w.sync_type == "semaphore"]
                    other = [w for w in ow if w.sync_type != "semaphore"]
                    keep = max(cap - len(other), 0)
                    hoist = sem_waits[:len(sem_waits) - keep] if keep else sem_waits
                    kept = sem_waits[len(hoist):] + other
                    assert len(kept) <= cap, (len(kept), cap, ins.opcode)
                    changed = True
                    for w in hoist:
                        n += 1
                        nop = mybir.InstNoOp(
                            name=f"wsplit-{n}",
                            sync_info=mybir.SyncInfo(on_wait=[w], on_update=[]),
                            bass_nofuse=True,
                            engine=ins.engine,
                        )
                        nc.register_instruction(nop, overwrite=True)
                        out.append(nop)
                    si.on_wait = kept
                out.append(ins)
            if changed:
                bb.instructions = out
    return nc


def _build():
    import concourse.bass as bass
    import concourse.mybir as mybir
    import concourse.tile as tile

    f32 = mybir.dt.float32
    bf16 = mybir.dt.bfloat16
    AF = mybir.ActivationFunctionType
    ALU = mybir.AluOpType
    AX = mybir.AxisListType

    from concourse.vector_clock import ScopedClock

    class _TrimTailTC(tile.TileContext):
        # Drop the second kernel-tail all-engine barrier: it only orders
        # the semaphore resets against engine halt, and nothing executes
        # after it. The first barrier (before resets) is kept, so resets
        # still happen on a quiesced machine and re-execution stays safe.
        def _drain_and_barrier(self, tick_clock, wait_clock):
            drain_inst = self.nc.sync.drain()
            wait_clock.add_sem_waits(
                drain_inst.ins, ScopedClock({None: tick_clock.global_clock})
            )
            self.nc.all_engine_barrier()
            assert self.sems is not None
            popped = self.nc._tile_sem_poison_stack.pop()
            assert popped is self._sem_poison
            self.nc.clear_and_free_semaphores(
                list(self.sems.allocated().values())
            )

    nc = bass.Bass()

    d_embT = nc.dram_tensor("embT", [128, KD * T], bf16, kind="ExternalInput")
    d_emb = nc.dram_tensor("emb", [T, D], bf16, kind="ExternalInput")
    d_cf = nc.dram_tensor("cf", [128, NCF], f32, kind="ExternalInput")
    d_cb = nc.dram_tensor("cb", [128, NCB], bf16, kind="ExternalInput")
    d_dxT = nc.dram_tensor("dxT", [128, KD * N], bf16, kind="ExternalInput")
    d_dyT = nc.dram_tensor("dyT", [128, KD * N], bf16, kind="ExternalInput")
    d_eT = nc.dram_tensor("eT", [128, KN * D], bf16, kind="ExternalInput")
    d_out = nc.dram_tensor("out", [T, D], f32, kind="ExternalOutput")

    with _TrimTailTC(nc) as tc, ExitStack() as ctx:
        work = ctx.enter_context(tc.tile_pool(name="work", bufs=1))
        stats = ctx.enter_context(tc.tile_pool(name="stats", bufs=1))
        p_u = ctx.enter_context(tc.tile_pool(name="p_u", bufs=2, space="PSUM"))
        p_wu = ctx.enter_context(tc.tile_pool(name="p_wu", bufs=1, space="PSUM"))
        p_sq = ctx.enter_context(tc.tile_pool(name="p_sq", bufs=2, space="PSUM"))
        p_g = ctx.enter_context(tc.tile_pool(name="p_g", bufs=2, space="PSUM"))
        p_med = ctx.enter_context(tc.tile_pool(name="p_med", bufs=1,
                                               space="PSUM"))

        # ---- PE dummy-feed operand: random bf16 (all-zero operands leave
        # the activity monitor cold -> clock gate never lifts).
        wu_sb = work.tile([128, 512], bf16)
        if SIM_MODE:
            nc.vector.memset(wu_sb[:], 1.0)
        else:
            nc.vector.random(wu_sb[:])
        wu_ps = p_wu.tile([128, 512], f32, tag="wu")

        def dummy(cols=512):
            nc.tensor.matmul(wu_ps[:, 0:cols], wu_sb[:, 0:128],
                             wu_sb[:, 0:cols], start=True, stop=True)

        # ---- ACT table preload: one Ln at boot loads the
        # natural_log_exp_and_others set; every later ACT func is in-set.
        pre_sb = stats.tile([1, 1], f32)
        nc.gpsimd.memset(pre_sb[:], 1.0)
        pre_o = stats.tile([1, 1], f32)
        nc.scalar.activation(pre_o[:], pre_sb[:], AF.Ln)

        # ---- negones row for the -Q_t broadcast (no DMA needed)
        negones_sb = work.tile([1, T], bf16)
        nc.gpsimd.memset(negones_sb[:], -1.0)

        # ---- DMAs: qSP-HWDGE executes these in FIFO order, each striped
        # across the 16 SDMA engines at ~full HBM rate. Order = the compute
        # pipeline's start schedule.
        embT_sb = work.tile([128, KD * T], bf16)
        nc.sync.dma_start(embT_sb[:], d_embT[:])
        emb_sb = work.tile([T, D], bf16)
        nc.sync.dma_start(emb_sb[:], d_emb[:])
        cf_sb = work.tile([128, NCF], f32)
        nc.sync.dma_start(cf_sb[:], d_cf[:])
        cb_sb = work.tile([128, NCB], bf16)
        nc.sync.dma_start(cb_sb[:], d_cb[:])
        trik_sb = cf_sb[:, 0:T]
        ident_sb = cf_sb[:, T:2 * T]
        xdvec_sb = cf_sb[:, 2 * T:2 * T + 1]
        utones_sb = cb_sb[:, 0:T]
        ustrict_sb = cb_sb[:, T:2 * T]
        dmaskT_sb = cb_sb[:, 2 * T:3 * T]

        dxT_sb = work.tile([128, KD * N], bf16)
        for j in range(NJ):
            nc.sync.dma_start(dxT_sb[:, j * 1024:(j + 1) * 1024],
                              d_dxT[:, j * 1024:(j + 1) * 1024])
        dyT_sb = work.tile([128, KD * N], bf16)
        for j in range(NJ):
            nc.sync.dma_start(dyT_sb[:, j * 1024:(j + 1) * 1024],
                              d_dyT[:, j * 1024:(j + 1) * 1024])
        eT_sb = work.tile([128, KN * D], bf16)
        for g in range(4):
            nc.sync.dma_start(eT_sb[:, g * 1024:(g + 1) * 1024],
                              d_eT[:, g * 1024:(g + 1) * 1024])

        # ---- all-ACT layernorm (zero cross-engine handoffs; use where the
        # DVE is busy and ACT idle).
        def ln_act(src, dst, tagp):
            junk = work.tile([T, D], bf16, tag="lnjunk")
            msum = stats.tile([T, 1], f32, tag=f"{tagp}_ms")
            nc.scalar.activation(junk[:], src, AF.Copy, accum_out=msum[:])
            negm = stats.tile([T, 1], f32, tag=f"{tagp}_nm")
            nc.scalar.mul(negm[:], msum[:], -1.0 / D)
            ssum = stats.tile([T, 1], f32, tag=f"{tagp}_ss")
            nc.scalar.activation(junk[:], src, AF.Square, bias=negm[:],
                                 accum_out=ssum[:])
            veps = stats.tile([T, 1], f32, tag=f"{tagp}_ve")
            nc.scalar.activation(veps[:], ssum[:], AF.Copy, scale=1.0 / D,
                                 bias=LN_EPS)
            lv = stats.tile([T, 1], f32, tag=f"{tagp}_lv")
            nc.scalar.activation(lv[:], veps[:], AF.Ln)
            rstd = stats.tile([T, 1], f32, tag=f"{tagp}_rs")
            nc.scalar.activation(rstd[:], lv[:], AF.Exp, scale=-0.5)
            nmr = stats.tile([T, 1], f32, tag=f"{tagp}_nr")
            nc.scalar.mul(nmr[:], negm[:], rstd[:])
            nc.scalar.activation(dst[:], src, AF.Identity,
                                 scale=rstd[:], bias=nmr[:])

        # ---- hybrid layernorm: DVE stats, ACT normalize. Shortest latency;
        # use on the critical path when both engines are free.
        def ln_fast(src, dst, tagp):
            stat6 = stats.tile([T, 6], f32, tag=f"{tagp}_s6")
            nc.vector.bn_stats(stat6[:], src)
            mv = stats.tile([T, 2], f32, tag=f"{tagp}_mv")
            nc.vector.bn_aggr(mv[:], stat6[:])
            veps = stats.tile([T, 1], f32, tag=f"{tagp}_ve")
            nc.vector.tensor_scalar_add(veps[:], mv[:, 1:2], LN_EPS)
            lv = stats.tile([T, 1], f32, tag=f"{tagp}_lv")
            nc.scalar.activation(lv[:], veps[:], AF.Ln)
            rstd = stats.tile([T, 1], f32, tag=f"{tagp}_rs")
            nc.scalar.activation(rstd[:], lv[:], AF.Exp, scale=-0.5)
            nmr = stats.tile([T, 1], f32, tag=f"{tagp}_nr")
            nc.vector.scalar_tensor_tensor(nmr[:], mv[:, 0:1], -1.0, rstd[:],
                                           op0=ALU.mult, op1=ALU.mult)
            nc.scalar.activation(dst[:], src, AF.Identity,
                                 scale=rstd[:], bias=nmr[:])

        # ---- u = relu(emb @ Dx.T): t-major, bf16 evac + row sums on DVE.
        # PE padded with dummies so the DVFS ramp never resets while the
        # dxT pieces stream in.
        u_sb = work.tile([T, N], bf16)
        su_part = stats.tile([T, NJ], f32)
        dummy(); dummy(); dummy()
        for j in range(NJ):
            ps = p_u.tile([128, 512], f32, tag="pu")
            for k in range(KD):
                nc.tensor.matmul(
                    ps[:],
                    embT_sb[:, k * T:(k + 1) * T],
                    dxT_sb[:, j * 1024 + k * 512: j * 1024 + (k + 1) * 512],
                    start=(k == 0),
                    stop=(k == KD - 1),
                )
            nc.vector.tensor_scalar(
                u_sb[:, j * 512:(j + 1) * 512], ps[:], 0.0, 0.0,
                op0=ALU.max, op1=ALU.add, accum_out=su_part[:, j:j + 1],
            )
            dummy()

        # ---- vn = LN(emb): off critical path, all-ACT while DVE evacs u.
        vn_sb = work.tile([T, D], bf16)
        ln_act(emb_sb[:], vn_sb, "vn")

        # ---- C^T coefficient matrix (f32 chain, bf16 product inputs) -----
        su = stats.tile([T, 1], f32)
        nc.vector.tensor_reduce(su[:], su_part[:], axis=AX.X, op=ALU.add)
        q_sb = stats.tile([T, 1], bf16)
        nc.scalar.activation(q_sb[:], su[:], AF.Ln, scale=C2, bias=xdvec_sb[:])
        dummy()
        qr = p_sq.tile([T, T], f32, tag="sq")               # Q_t inclusive row
        nc.tensor.matmul(qr[0:1, :], q_sb[:], utones_sb[:], start=True,
                         stop=True)
        colsc = p_sq.tile([T, T], f32, tag="sq")            # Q_s - q_s column
        nc.tensor.matmul(colsc[:, 0:1], ustrict_sb[:], q_sb[:], start=True,
                         stop=True)
        qr_sb = stats.tile([1, T], bf16)
        nc.vector.tensor_copy(qr_sb[:], qr[0:1, :])
        colsc_sb = stats.tile([T, 1], f32)
        nc.vector.tensor_copy(colsc_sb[:], colsc[:, 0:1])
        dummy()
        bc = p_sq.tile([T, T], f32, tag="sq")               # [s,t] = -Q_t
        nc.tensor.matmul(bc[:], negones_sb[:], qr_sb[:], start=True, stop=True)
        expo = work.tile([T, T], f32)
        nc.vector.scalar_tensor_tensor(
            expo[:], bc[:], colsc_sb[:], trik_sb[:], op0=ALU.add, op1=ALU.add
        )
        expoc = work.tile([T, T], f32)
        nc.vector.tensor_scalar_max(expoc[:], expo[:], -80.0)
        ct_sb = work.tile([T, T], bf16)                     # C^T [s,t]
        nc.scalar.activation(ct_sb[:], expoc[:], AF.Exp)
        dummy(); dummy()

        # ---- X^T directly (n-major): XT_c = u_c^T(stationary) @ C^T;
        # Gram super-chunks G_q = sum_c XT_c XT_c^T; a* += (G_q.dmask) @ vn.
        xt_sb = work.tile([128, N], bf16)
        aps = p_med.tile([T, D], f32, tag="med")
        gq_ps = {}

        def xt_block(q):
            for cc in range(4):
                c = 4 * q + cc
                tp = p_sq.tile([128, T], f32, tag="sq")
                nc.tensor.matmul(tp[:], u_sb[:, c * T:(c + 1) * T], ct_sb[:],
                                 start=True, stop=True)
                # GPSIMD cannot read PSUM; alternate DVE / ACT for evacs.
                if c % 2 == 0:
                    nc.vector.tensor_copy(xt_sb[:, c * T:(c + 1) * T], tp[:])
                else:
                    nc.scalar.copy(xt_sb[:, c * T:(c + 1) * T], tp[:])

        def g_block(q):
            g = p_g.tile([T, T], f32, tag="g")
            gq_ps[q] = g
            for cc in range(4):
                c = 4 * q + cc
                nc.tensor.matmul(g[:], xt_sb[:, c * T:(c + 1) * T],
                                 xt_sb[:, c * T:(c + 1) * T],
                                 start=(cc == 0), stop=(cc == 3))

        def astar_block(q):
            g = gq_ps.pop(q)
            wt = work.tile([T, T], bf16, tag=f"wt{q % 2}")
            nc.vector.tensor_tensor(wt[:], g[:], dmaskT_sb[:], op=ALU.mult)
            nc.tensor.matmul(aps[:], wt[:], vn_sb[:], start=(q == 0),
                             stop=(q == 3))

        xt_block(0)
        xt_block(1)
        g_block(0)
        xt_block(2)
        g_block(1)
        astar_block(0)
        xt_block(3)
        g_block(2)
        astar_block(1)
        g_block(3)
        astar_block(2)
        astar_block(3)

        # ---- lna = LN(a*) (f32 for the PE transposes), lnaT bf16 ----------
        lna_sb = work.tile([T, D], f32)
        ln_fast(aps[:], lna_sb, "la")
        dummy(); dummy()
        lnaT_sb = work.tile([128, KD * T], bf16)
        for k in range(KD):
            tp = p_sq.tile([T, T], f32, tag="sq")
            nc.tensor.transpose(tp[:], lna_sb[:, k * T:(k + 1) * T], ident_sb)
            if k == 0:
                nc.vector.tensor_copy(lnaT_sb[:, k * T:(k + 1) * T], tp[:])
            else:
                nc.scalar.copy(lnaT_sb[:, k * T:(k + 1) * T], tp[:])

        # ---- Ycore^T directly (n-major): yc_c = Dy_c(stationary) @ lnaT;
        # yT_c = relu(yc_c) * xT_c on DVE/GPSIMD; vraw += yT_c^T @ eT_c.
        # PE order has one chunk of lookahead so the STT evac of chunk c
        # hides under the ycoreT matmuls of chunk c+1.
        yt_sb = work.tile([128, N], bf16)
        vps = p_med.tile([T, D], f32, tag="med")
        yc_ps = {}

        def yc_block(c):
            j = c // 4
            r = c % 4
            ps = p_sq.tile([128, T], f32, tag="sq")
            yc_ps[c] = ps
            for k in range(KD):
                nc.tensor.matmul(
                    ps[:],
                    dyT_sb[:, j * 1024 + k * 512 + r * T:
                           j * 1024 + k * 512 + (r + 1) * T],
                    lnaT_sb[:, k * T:(k + 1) * T],
                    start=(k == 0),
                    stop=(k == KD - 1),
                )

        def yt_evac(c):
            ps = yc_ps.pop(c)
            nc.vector.scalar_tensor_tensor(
                yt_sb[:, c * T:(c + 1) * T], ps[:], 0.0,
                xt_sb[:, c * T:(c + 1) * T], op0=ALU.max, op1=ALU.mult,
            )

        def vraw_block(c):
            nc.tensor.matmul(vps[:], yt_sb[:, c * T:(c + 1) * T],
                             eT_sb[:, c * D:(c + 1) * D],
                             start=(c == 0), stop=(c == KN - 1))

        yc_block(0)
        yt_evac(0)
        yc_block(1)
        yt_evac(1)
        for c in range(2, KN):
            yc_block(c)
            yt_evac(c)
            vraw_block(c - 2)
        vraw_block(KN - 2)
        vraw_block(KN - 1)

        # ---- v* = LN(vraw), f32 out, DMA back ----------------------------
        vstar_sb = work.tile([T, D], f32)
        ln_fast(vps[:], vstar_sb, "vs")
        nc.sync.dma_start(d_out[:], vstar_sb[:])

    return _split_multiwait(nc, mybir)


def _numpy_fallback(embeddings, E, Dx, Dy, x_state, rho_state):
    # General-path reference (only used if initial states are nonzero).
    def ln(x):
        m = x.mean(-1, keepdims=True)
        v = ((x - m) ** 2).mean(-1, keepdims=True)
        return (x - m) / np.sqrt(v + LN_EPS)

    x_s = x_state.astype(np.float32).copy()
    rho = rho_state.astype(np.float32).copy()
    outs = np.zeros((B, T, D), dtype=np.float32)
    for t in range(T):
        v_prev = embeddings[:, t, :]
        x_upd = np.maximum(v_prev @ Dx.T, 0.0)
        x_t = XD * x_s + x_upd
        x_t = x_t / np.maximum(np.abs(x_t).sum(-1, keepdims=True), L1_EPS)
        a_star = np.einsum("bdn,bn->bd", rho, x_t)
        y_core = ln(a_star) @ Dy.T
        y_t = np.maximum(y_core, 0.0) * np.maximum(x_t, 0.0)
        outs[:, t, :] = ln(y_t @ E.T)
        vn = ln(v_prev)
        rho = UD * rho + np.einsum("bd,bn->bdn", vn, x_t)
        x_s = x_t
    return outs


def kernel(embeddings, E, Dx, Dy, x_state, rho_state):
    embeddings = np.ascontiguousarray(embeddings, dtype=np.float32)
    E = np.ascontiguousarray(E, dtype=np.float32)
    Dx = np.ascontiguousarray(Dx, dtype=np.float32)
    Dy = np.ascontiguousarray(Dy, dtype=np.float32)

    if np.any(x_state) or np.any(rho_state):
        return _numpy_fallback(embeddings, E, Dx, Dy,
                               np.asarray(x_state, np.float32),
                               np.asarray(rho_state, np.float32))

    from concourse.bass_utils import run_bass_kernel_spmd

    if "nc" not in _cache:
        _cache["nc"] = _build()
    nc = _cache["nc"]

    cf = _consts_f()
    cb = _consts_b()
    dxT = _pack_jk(Dx.T.reshape(KD, 128, N)).astype(BF16)
    dyT = _pack_jk(Dy.T.reshape(KD, 128, N)).astype(BF16)
    eT = np.ascontiguousarray(
        E.T.reshape(KN, 128, D).transpose(1, 0, 2).reshape(128, KN * D)
    ).astype(BF16)

    in_maps = []
    for b in range(B):
        emb_b = embeddings[b]
        embT_b = np.ascontiguousarray(
            emb_b.T.reshape(KD, 128, T).transpose(1, 0, 2).reshape(128, KD * T)
        ).astype(BF16)
        in_maps.append({
            "embT": embT_b,
            "emb": emb_b.astype(BF16),
            "cf": cf,
            "cb": cb,
            "dxT": dxT,
            "dyT": dyT,
            "eT": eT,
        })

    res = run_bass_kernel_spmd(nc, in_maps, list(range(B)))
    _cache["last_results"] = res
    return np.stack([res.results[i]["out"] for i in range(B)])


# revision 15
# speedup vs baseline: 1.2910x; 1.1888x over previous
"""Trainium2 Bass kernel for the BDH-style recurrent block.

Strategy: data-parallel over B (8 batches -> 8 NeuronCores, no collectives).
The T=128-step scan is de-sequentialized into dense matmuls per core:

  u_t = relu(emb_t @ Dx.T)                                  (T,N)
  x_t = (XD*x_{t-1} + u_t)/s_t  with s_t = XD + sum(u_t)    (L1 norm; x>=0)
      => x = C @ u, C[t,s] = (1/s_s) exp(A_t - A_s), A_t = cumsum log(XD/s_r)
  a*_t = rho_{t-1} @ x_t = ((DecayMask . X X^T) @ ln(emb))_t   (rho_0 = 0)
  y_t  = relu(ln(a*_t) @ Dy.T) * x_t                        (x_t >= 0)
  v*_t = ln(y_t @ E.T)

All matmul operands are bf16 (f32 PSUM accumulation): halves the weight DMA
(HBM streaming is the dominant cost) and streams 1 col/cycle at any free
dim. The dataflow is transpose-free: X^T and Ycore^T are produced directly
in n-major layout by swapping the stationary operand. a* accumulates per
512-column Gram super-chunk, removing the full-Gram barrier.

The C matrix is built WITHOUT the -Q_t column broadcast: C'[s,t] =
exp(trik[s,t] + (Q_s - q_s)) is one ACT op (per-partition bias). The
dropped diag(e^{-Q_t}) row factor is absorbed exactly: e^{-Q_s} folds into
vn's LN output scale, and the LN eps is replaced per-row by eps*e^{2Q_t}
(LN of c*v with eps*c^2 equals LN of v with eps — exact identity), so the
a*/vraw row scales cancel inside the downstream LayerNorms.

Every ACT function is chosen from the one 'natural_log_exp_and_others'
table (rsqrt = exp(-.5 ln)), so the 1.3us table reload is paid once at
boot and never again.
"""

import math
from contextlib import ExitStack

import numpy as np
import ml_dtypes

BF16 = ml_dtypes.bfloat16

N = 2048
D = 256
B = 8
T = 128
XD = 0.97
UD = 0.97
LN_EPS = 1e-5
L1_EPS = 1e-12

# log-domain recentring: E[sum relu(N(0,1)) over 2048] + XD ~ 818.9
LNC2INV = 6.7065
C2 = math.exp(-LNC2INV)
K1 = LNC2INV - math.log(XD)
TRIK_FLOOR = -60.0   # exp(-60+colsc) ~ 1e-24 << any real C entry; in-table

KD = D // 128   # 2
KN = N // 128   # 16
NJ = N // 512   # 4

NCF = 2 * T + 1   # f32 consts: trik | ident | xdvec
NCBIG = 6 * T     # bf16 block: embT(2T) | emb(D) | ustrict(T) | dmaskT(T)

_cache = {}


def _consts_f():
    r = np.arange(T)
    tri = r[None, :] - r[:, None]                                   # [s,t] t-s
    trik = np.where(tri >= 0,
                    np.maximum(-K1 * tri - LNC2INV, TRIK_FLOOR),
                    TRIK_FLOOR).astype(np.float32)
    ident = np.eye(T, dtype=np.float32)
    xdvec = np.full((T, 1), C2 * XD, dtype=np.float32)
    xdvec[0, 0] = 0.0                                               # x_{-1} = 0
    return np.ascontiguousarray(np.concatenate([trik, ident, xdvec], axis=1))


def _consts_b():
    r = np.arange(T)
    ustrict = (r[:, None] < r[None, :]).astype(BF16)                # [r,t] r<t
    pw = r[:, None] - 1 - r[None, :]                                # [t,s] t-1-s
    dmask = np.where(pw >= 0, UD ** np.maximum(pw, 0), 0.0)
    dmaskT = np.ascontiguousarray(dmask.T).astype(BF16)             # [s,t]
    return np.ascontiguousarray(np.concatenate([ustrict, dmaskT], axis=1))


def _pack_jk(wT):
    # (KD,128,N) k-major -> (128, [j(4), k(2), 512]) per-partition contiguous
    return np.ascontiguousarray(
        wT.reshape(KD, 128, NJ, 512).transpose(1, 2, 0, 3).reshape(128, KD * N))


def _split_multiwait(nc, mybir):
    """This walrus build caps sync waits per instruction (1 for regular
    instructions, 2 for EventSemaphore). Tile attaches more (e.g. the
    kernel-tail Drain waits on every live semaphore). Hoist excess waits
    onto same-engine NOPs placed immediately before the instruction —
    engine queues are sequential, so semantics are preserved."""
    n = 0
    for f in nc.m.functions:
        for bb in f.blocks:
            out = []
            changed = False
            for ins in bb.instructions:
                si = ins.sync_info
                ow = list(si.on_wait) if si is not None else []
                cap = 2 if ins.opcode == "EventSemaphore" else 1
                if len(ow) > cap:
                    sem_waits = [w for w in ow if w.sync_type == "semaphore"]
                    other = [w for w in ow if w.sync_type != "semaphore"]
                    keep = max(cap - len(other), 0)
                    hoist = sem_waits[:len(sem_waits) - keep] if keep else sem_waits
                    kept = sem_waits[len(hoist):] + other
                    assert len(kept) <= cap, (len(kept), cap, ins.opcode)
                    changed = True
                    for w in hoist:
                        n += 1
                        nop = mybir.InstNoOp(
                            name=f"wsplit-{n}",
                            sync_info=mybir.SyncInfo(on_wait=[w], on_update=[]),
                            bass_nofuse=True,
                            engine=ins.engine,
                        )
                        nc.register_instruction(nop, overwrite=True)
                        out.append(nop)
                    si.on_wait = kept
                out.append(ins)
            if changed:
                bb.instructions = out
    return nc


def _build():
    import concourse.bass as bass
    import concourse.mybir as mybir
    import concourse.tile as tile

    f32 = mybir.dt.float32
    bf16 = mybir.dt.bfloat16
    AF = mybir.ActivationFunctionType
    ALU = mybir.AluOpType
    AX = mybir.AxisListType

    from concourse.vector_clock import ScopedClock

    class _TrimTailTC(tile.TileContext):
        # Drop the second kernel-tail all-engine barrier: it only orders
        # the semaphore resets against engine halt, and nothing executes
        # after it. The first barrier (before resets) is kept, so resets
        # still happen on a quiesced machine and re-execution stays safe.
        def _drain_and_barrier(self, tick_clock, wait_clock):
            drain_inst = self.nc.sync.drain()
            wait_clock.add_sem_waits(
                drain_inst.ins, ScopedClock({None: tick_clock.global_clock})
            )
            self.nc.all_engine_barrier()
            assert self.sems is not None
            popped = self.nc._tile_sem_poison_stack.pop()
            assert popped is self._sem_poison
            self.nc.clear_and_free_semaphores(
                list(self.sems.allocated().values())
            )

    nc = bass.Bass()

    d_cbig = nc.dram_tensor("cbig", [128, NCBIG], bf16, kind="ExternalInput")
    d_cf = nc.dram_tensor("cf", [128, NCF], f32, kind="ExternalInput")
    d_dxT = nc.dram_tensor("dxT", [128, KD * N], bf16, kind="ExternalInput")
    d_dyT = nc.dram_tensor("dyT", [128, KD * N], bf16, kind="ExternalInput")
    d_eT = nc.dram_tensor("eT", [128, KN * D], bf16, kind="ExternalInput")
    d_out = nc.dram_tensor("out", [T, D], f32, kind="ExternalOutput")

    with _TrimTailTC(nc) as tc, ExitStack() as ctx:
        work = ctx.enter_context(tc.tile_pool(name="work", bufs=1))
        stats = ctx.enter_context(tc.tile_pool(name="stats", bufs=1))
        p_u = ctx.enter_context(tc.tile_pool(name="p_u", bufs=2, space="PSUM"))
        p_sq = ctx.enter_context(tc.tile_pool(name="p_sq", bufs=3, space="PSUM"))
        p_g = ctx.enter_context(tc.tile_pool(name="p_g", bufs=2, space="PSUM"))
        p_med = ctx.enter_context(tc.tile_pool(name="p_med", bufs=1,
                                               space="PSUM"))

        # ---- ACT table preload: one Ln at boot loads the
        # natural_log_exp_and_others set; every later ACT func is in-set.
        pre_sb = stats.tile([1, 1], f32)
        nc.gpsimd.memset(pre_sb[:], 1.0)
        pre_o = stats.tile([1, 1], f32)
        nc.scalar.activation(pre_o[:], pre_sb[:], AF.Ln)

        # const bias APs (walrus needs non-Copy act biases as APs)
        c_eps = stats.tile([T, 1], f32)
        nc.gpsimd.memset(c_eps[:], LN_EPS)
        c_lneps = stats.tile([T, 1], f32)
        nc.gpsimd.memset(c_lneps[:], math.log(LN_EPS))

        # ---- DMAs: qSP-HWDGE executes these in FIFO order, each striped
        # across the 16 SDMA engines at ~full HBM rate. Few, large pieces.
        cbig_sb = work.tile([128, NCBIG], bf16)
        nc.sync.dma_start(cbig_sb[:], d_cbig[:])
        cf_sb = work.tile([128, NCF], f32)
        nc.sync.dma_start(cf_sb[:], d_cf[:])
        embT_sb = cbig_sb[:, 0:2 * T]
        emb_sb = cbig_sb[:, 2 * T:2 * T + D]
        ustrict_sb = cbig_sb[:, 2 * T + D:3 * T + D]
        dmaskT_sb = cbig_sb[:, 3 * T + D:4 * T + D]
        trik_sb = cf_sb[:, 0:T]
        ident_sb = cf_sb[:, T:2 * T]
        xdvec_sb = cf_sb[:, 2 * T:2 * T + 1]

        dxT_sb = work.tile([128, KD * N], bf16)
        for h in range(2):
            nc.sync.dma_start(dxT_sb[:, h * 2048:(h + 1) * 2048],
                              d_dxT[:, h * 2048:(h + 1) * 2048])
        dyT_sb = work.tile([128, KD * N], bf16)
        for h in range(2):
            nc.sync.dma_start(dyT_sb[:, h * 2048:(h + 1) * 2048],
                              d_dyT[:, h * 2048:(h + 1) * 2048])
        eT_sb = work.tile([128, KN * D], bf16)
        for h in range(2):
            nc.sync.dma_start(eT_sb[:, h * 2048:(h + 1) * 2048],
                              d_eT[:, h * 2048:(h + 1) * 2048])

        # ---- u = relu(emb @ Dx.T): t-major; each chunk's evac is split
        # DVE/ACT half-and-half so the tail chunk's latency halves.
        u_sb = work.tile([T, N], bf16)
        su_part = stats.tile([T, 2 * NJ], f32)
        for j in range(NJ):
            ps = p_u.tile([128, 512], f32, tag="pu")
            for k in range(KD):
                nc.tensor.matmul(
                    ps[:],
                    embT_sb[:, k * T:(k + 1) * T],
                    dxT_sb[:, j * 1024 + k * 512: j * 1024 + (k + 1) * 512],
                    start=(k == 0),
                    stop=(k == KD - 1),
                )
            nc.vector.tensor_scalar(
                u_sb[:, j * 512:j * 512 + 256], ps[:, 0:256], 0.0, 0.0,
                op0=ALU.max, op1=ALU.add, accum_out=su_part[:, 2 * j:2 * j + 1],
            )
            nc.scalar.activation(
                u_sb[:, j * 512 + 256:(j + 1) * 512], ps[:, 256:512], AF.Relu,
                accum_out=su_part[:, 2 * j + 1:2 * j + 2],
            )

        # ---- vn stats (all-ACT, runs while DVE evacs u) ------------------
        junk = work.tile([T, D], bf16)
        msum = stats.tile([T, 1], f32)
        nc.scalar.activation(junk[:], emb_sb, AF.Copy, accum_out=msum[:])
        negm = stats.tile([T, 1], f32)
        nc.scalar.mul(negm[:], msum[:], -1.0 / D)
        ssum = stats.tile([T, 1], f32)
        nc.scalar.activation(junk[:], emb_sb, AF.Square, bias=negm[:],
                             accum_out=ssum[:])
        lv_vn = stats.tile([T, 1], f32)
        nc.scalar.activation(lv_vn[:], ssum[:], AF.Ln, scale=1.0 / D,
                             bias=c_eps[:])
        rstd_vn = stats.tile([T, 1], f32)
        nc.scalar.activation(rstd_vn[:], lv_vn[:], AF.Exp, scale=-0.5)
        nmr_vn = stats.tile([T, 1], f32)
        nc.scalar.mul(nmr_vn[:], negm[:], rstd_vn[:])

        # ---- C' matrix: q -> strict cumsum -> one Exp with bias ----------
        su = stats.tile([T, 1], f32)
        nc.vector.tensor_reduce(su[:], su_part[:], axis=AX.X, op=ALU.add)
        q_sb = stats.tile([T, 1], bf16)
        nc.scalar.activation(q_sb[:], su[:], AF.Ln, scale=C2, bias=xdvec_sb[:])
        colsc = p_sq.tile([T, T], f32, tag="sq")            # Q_s - q_s column
        nc.tensor.matmul(colsc[:, 0:1], ustrict_sb[:], q_sb[:], start=True,
                         stop=True)
        colsc_sb = stats.tile([T, 1], f32)
        nc.vector.tensor_copy(colsc_sb[:], colsc[:, 0:1])
        ct_sb = work.tile([T, T], bf16)                     # C'[s,t]
        nc.scalar.activation(ct_sb[:], trik_sb[:], AF.Exp, bias=colsc_sb[:])

        # ---- scale-absorption factors (off critical path) ----------------
        qq = stats.tile([T, 1], f32)                        # Q_s inclusive
        nc.vector.tensor_tensor(qq[:], colsc[:, 0:1], q_sb[:], op=ALU.add)
        expQ = stats.tile([T, 1], f32)                      # e^{-Q_s}
        nc.scalar.activation(expQ[:], qq[:], AF.Exp, scale=-1.0)
        epsQ = stats.tile([T, 1], f32)                      # eps * e^{2 Q_t}
        nc.scalar.activation(epsQ[:], qq[:], AF.Exp, scale=2.0,
                             bias=c_lneps[:])
        rstd2 = stats.tile([T, 1], f32)
        nc.scalar.mul(rstd2[:], rstd_vn[:], expQ[:])
        nmr2 = stats.tile([T, 1], f32)
        nc.scalar.mul(nmr2[:], nmr_vn[:], expQ[:])
        vn_sb = work.tile([T, D], bf16)                     # e^{-Q} * LN(emb)
        nc.scalar.activation(vn_sb[:], emb_sb, AF.Identity,
                             scale=rstd2[:], bias=nmr2[:])

        # ---- X^T directly (n-major): XT_c = u_c^T(stationary) @ C'^T;
        # Gram super-chunks G_q = sum_c XT_c XT_c^T; a* += (G_q.dmask) @ vn.
        xt_sb = work.tile([128, N], bf16)
        aps = p_med.tile([T, D], f32, tag="med")
        gq_ps = {}

        def xt_block(q):
            for cc in range(4):
                c = 4 * q + cc
                tp = p_sq.tile([128, T], f32, tag="sq")
                nc.tensor.matmul(tp[:], u_sb[:, c * T:(c + 1) * T], ct_sb[:],
                                 start=True, stop=True)
                # GPSIMD cannot read PSUM; alternate DVE / ACT for evacs.
                if c % 2 == 0:
                    nc.vector.tensor_copy(xt_sb[:, c * T:(c + 1) * T], tp[:])
                else:
                    nc.scalar.copy(xt_sb[:, c * T:(c + 1) * T], tp[:])

        def g_block(q):
            g = p_g.tile([T, T], f32, tag="g")
            gq_ps[q] = g
            for cc in range(4):
                c = 4 * q + cc
                nc.tensor.matmul(g[:], xt_sb[:, c * T:(c + 1) * T],
                                 xt_sb[:, c * T:(c + 1) * T],
                                 start=(cc == 0), stop=(cc == 3))

        def astar_block(q):
            g = gq_ps.pop(q)
            wt = work.tile([T, T], bf16, tag=f"wt{q % 2}")
            nc.vector.tensor_tensor(wt[:], g[:], dmaskT_sb[:], op=ALU.mult)
            nc.tensor.matmul(aps[:], wt[:], vn_sb[:], start=(q == 0),
                             stop=(q == 3))

        xt_block(0)
        xt_block(1)
        g_block(0)
        xt_block(2)
        g_block(1)
        astar_block(0)
        xt_block(3)
        g_block(2)
        astar_block(1)
        g_block(3)
        astar_block(2)
        astar_block(3)

        # ---- layernorm, 5 links: DVE stats -> ACT ln/exp -> DVE normalize.
        # eps arrives as a per-row AP so the absorbed e^{Q_t} row scale
        # cancels exactly (LN(c*v, eps*c^2) == LN(v, eps)).
        def ln_fast(src, dst, tagp, eps_ap):
            stat6 = stats.tile([T, 6], f32, tag=f"{tagp}_s6")
            nc.vector.bn_stats(stat6[:], src)
            mv = stats.tile([T, 2], f32, tag=f"{tagp}_mv")
            nc.vector.bn_aggr(mv[:], stat6[:])
            lv = stats.tile([T, 1], f32, tag=f"{tagp}_lv")
            nc.scalar.activation(lv[:], mv[:, 1:2], AF.Ln, bias=eps_ap)
            rstd = stats.tile([T, 1], f32, tag=f"{tagp}_rs")
            nc.scalar.activation(rstd[:], lv[:], AF.Exp, scale=-0.5)
            nc.vector.tensor_scalar(dst[:], src, mv[:, 0:1], rstd[:],
                                    op0=ALU.subtract, op1=ALU.mult)

        # ---- lna = LN(a*) (f32 for the PE transposes), lnaT bf16 ----------
        lna_sb = work.tile([T, D], f32)
        ln_fast(aps[:], lna_sb, "la", epsQ[:])
        lnaT_sb = work.tile([128, KD * T], bf16)
        for k in range(KD):
            tp = p_sq.tile([T, T], f32, tag="sq")
            nc.tensor.transpose(tp[:], lna_sb[:, k * T:(k + 1) * T], ident_sb)
            if k == 0:
                nc.vector.tensor_copy(lnaT_sb[:, k * T:(k + 1) * T], tp[:])
            else:
                nc.scalar.copy(lnaT_sb[:, k * T:(k + 1) * T], tp[:])

        # ---- Ycore^T directly (n-major): yc_c = Dy_c(stationary) @ lnaT;
        # yT_c = relu(yc_c) * xT_c on DVE; vraw += yT_c^T @ eT_c. One chunk
        # of PE lookahead hides each chunk's DVE evac.
        yt_sb = work.tile([128, N], bf16)
        vps = p_med.tile([T, D], f32, tag="med")
        yc_ps = {}

        def yc_block(c):
            j = c // 4
            r = c % 4
            ps = p_sq.tile([128, T], f32, tag="sq")
            yc_ps[c] = ps
            for k in range(KD):
                nc.tensor.matmul(
                    ps[:],
                    dyT_sb[:, j * 1024 + k * 512 + r * T:
                           j * 1024 + k * 512 + (r + 1) * T],
                    lnaT_sb[:, k * T:(k + 1) * T],
                    start=(k == 0),
                    stop=(k == KD - 1),
                )

        def yt_evac(c):
            ps = yc_ps.pop(c)
            nc.vector.scalar_tensor_tensor(
                yt_sb[:, c * T:(c + 1) * T], ps[:], 0.0,
                xt_sb[:, c * T:(c + 1) * T], op0=ALU.max, op1=ALU.mult,
            )

        def vraw_block(c):
            nc.tensor.matmul(vps[:], yt_sb[:, c * T:(c + 1) * T],
                             eT_sb[:, c * D:(c + 1) * D],
                             start=(c == 0), stop=(c == KN - 1))

        yc_block(0)
        yt_evac(0)
        yc_block(1)
        yt_evac(1)
        for c in range(2, KN):
            yc_block(c)
            yt_evac(c)
            vraw_block(c - 2)
        vraw_block(KN - 2)
        vraw_block(KN - 1)

        # ---- v* = LN(vraw) with the same per-row eps, f32 out, DMA back --
        vstar_sb = work.tile([T, D], f32)
        ln_fast(vps[:], vstar_sb, "vs", epsQ[:])
        nc.sync.dma_start(d_out[:], vstar_sb[:])

    return _split_multiwait(nc, mybir)


def _numpy_fallback(embeddings, E, Dx, Dy, x_state, rho_state):
    # General-path reference (only used if initial states are nonzero).
    def ln(x):
        m = x.mean(-1, keepdims=True)
        v = ((x - m) ** 2).mean(-1, keepdims=True)
        return (x - m) / np.sqrt(v + LN_EPS)

    x_s = x_state.astype(np.float32).copy()
    rho = rho_state.astype(np.float32).copy()
    outs = np.zeros((B, T, D), dtype=np.float32)
    for t in range(T):
        v_prev = embeddings[:, t, :]
        x_upd = np.maximum(v_prev @ Dx.T, 0.0)
        x_t = XD * x_s + x_upd
        x_t = x_t / np.maximum(np.abs(x_t).sum(-1, keepdims=True), L1_EPS)
        a_star = np.einsum("bdn,bn->bd", rho, x_t)
        y_core = ln(a_star) @ Dy.T
        y_t = np.maximum(y_core, 0.0) * np.maximum(x_t, 0.0)
        outs[:, t, :] = ln(y_t @ E.T)
        vn = ln(v_prev)
        rho = UD * rho + np.einsum("bd,bn->bdn", vn, x_t)
        x_s = x_t
    return outs


def kernel(embeddings, E, Dx, Dy, x_state, rho_state):
    embeddings = np.ascontiguousarray(embeddings, dtype=np.float32)
    E = np.ascontiguousarray(E, dtype=np.float32)
    Dx = np.ascontiguousarray(Dx, dtype=np.float32)
    Dy = np.ascontiguousarray(Dy, dtype=np.float32)

    if np.any(x_state) or np.any(rho_state):
        return _numpy_fallback(embeddings, E, Dx, Dy,
                               np.asarray(x_state, np.float32),
                               np.asarray(rho_state, np.float32))

    from concourse.bass_utils import run_bass_kernel_spmd

    if "nc" not in _cache:
        _cache["nc"] = _build()
    nc = _cache["nc"]

    cf = _consts_f()
    cb = _consts_b()
    dxT = _pack_jk(Dx.T.reshape(KD, 128, N)).astype(BF16)
    dyT = _pack_jk(Dy.T.reshape(KD, 128, N)).astype(BF16)
    eT = np.ascontiguousarray(
        E.T.reshape(KN, 128, D).transpose(1, 0, 2).reshape(128, KN * D)
    ).astype(BF16)

    in_maps = []
    for b in range(B):
        emb_b = embeddings[b]
        embT_b = np.ascontiguousarray(
            emb_b.T.reshape(KD, 128, T).transpose(1, 0, 2).reshape(128, KD * T)
        ).astype(BF16)
        cbig = np.ascontiguousarray(np.concatenate(
            [embT_b, emb_b.astype(BF16), cb], axis=1))
        in_maps.append({
            "cbig": cbig,
            "cf": cf,
            "dxT": dxT,
            "dyT": dyT,
            "eT": eT,
        })

    res = run_bass_kernel_spmd(nc, in_maps, list(range(B)))
    _cache["last_results"] = res
    return np.stack([res.results[i]["out"] for i in range(B)])


# revision 23
# speedup vs baseline: 1.3293x; 1.0297x over previous
"""Trainium2 Bass kernel for the BDH-style recurrent block.

Strategy: data-parallel over B (8 batches -> 8 NeuronCores, no collectives).
The T=128-step scan is de-sequentialized into dense matmuls per core:

  u_t = relu(emb_t @ Dx.T)                                  (T,N)
  x_t = (XD*x_{t-1} + u_t)/s_t  with s_t = XD + sum(u_t)    (L1 norm; x>=0)
      => x = C @ u, C[t,s] = (1/s_s) exp(A_t - A_s), A_t = cumsum log(XD/s_r)
  a*_t = rho_{t-1} @ x_t = ((DecayMask . X X^T) @ ln(emb))_t   (rho_0 = 0)
  y_t  = relu(ln(a*_t) @ Dy.T) * x_t                        (x_t >= 0)
  v*_t = ln(y_t @ E.T)

All matmul operands are bf16 (f32 PSUM accumulation): halves the weight DMA
(HBM streaming is the dominant cost) and streams 1 col/cycle at any free
dim. The dataflow is transpose-free: X^T and Ycore^T are produced directly
in n-major layout by swapping the stationary operand. a* accumulates per
512-column Gram super-chunk, removing the full-Gram barrier.

The C matrix is built WITHOUT the -Q_t column broadcast: C'[s,t] =
exp(trik[s,t] + (Q_s - q_s)) is one ACT op (per-partition bias). The
dropped diag(e^{-Q_t}) row factor is absorbed exactly: e^{-Q_s} folds into
vn's LN output scale, and the LN eps is replaced per-row by eps*e^{2Q_t}
(LN of c*v with eps*c^2 equals LN of v with eps — exact identity), so the
a*/vraw row scales cancel inside the downstream LayerNorms.

Every ACT function is chosen from the one 'natural_log_exp_and_others'
table (rsqrt = exp(-.5 ln)), so the 1.3us table reload is paid once at
boot and never again.
"""

import math
from contextlib import ExitStack

import numpy as np
import ml_dtypes

BF16 = ml_dtypes.bfloat16

N = 2048
D = 256
B = 8
T = 128
XD = 0.97
UD = 0.97
LN_EPS = 1e-5
L1_EPS = 1e-12

# log-domain recentring: E[sum relu(N(0,1)) over 2048] + XD ~ 818.9
LNC2INV = 6.7065
C2 = math.exp(-LNC2INV)
K1 = LNC2INV - math.log(XD)
TRIK_FLOOR = -60.0   # exp(-60+colsc) ~ 1e-24 << any real C entry; in-table

KD = D // 128   # 2
KN = N // 128   # 16
NJ = N // 512   # 4

NCF = 2 * T + 1   # f32 consts: trik | ident | xdvec
NCBIG = 6 * T     # bf16 block: embT(2T) | emb(D) | ustrict(T) | dmaskT(T)

_cache = {}


def _consts_f():
    r = np.arange(T)
    tri = r[None, :] - r[:, None]                                   # [s,t] t-s
    trik = np.where(tri >= 0,
                    np.maximum(-K1 * tri - LNC2INV, TRIK_FLOOR),
                    TRIK_FLOOR).astype(np.float32)
    ident = np.eye(T, dtype=np.float32)
    xdvec = np.full((T, 1), C2 * XD, dtype=np.float32)
    xdvec[0, 0] = 0.0                                               # x_{-1} = 0
    return np.ascontiguousarray(np.concatenate([trik, ident, xdvec], axis=1))


def _consts_b():
    r = np.arange(T)
    ustrict = (r[:, None] < r[None, :]).astype(BF16)                # [r,t] r<t
    pw = r[:, None] - 1 - r[None, :]                                # [t,s] t-1-s
    dmask = np.where(pw >= 0, UD ** np.maximum(pw, 0), 0.0)
    dmaskT = np.ascontiguousarray(dmask.T).astype(BF16)             # [s,t]
    return np.ascontiguousarray(np.concatenate([ustrict, dmaskT], axis=1))


def _pack_jk(wT):
    # (KD,128,N) k-major -> (128, [j(4), k(2), 512]) per-partition contiguous
    return np.ascontiguousarray(
        wT.reshape(KD, 128, NJ, 512).transpose(1, 2, 0, 3).reshape(128, KD * N))


def _split_multiwait(nc, mybir):
    """This walrus build caps sync waits per instruction (1 for regular
    instructions, 2 for EventSemaphore). Tile attaches more (e.g. the
    kernel-tail Drain waits on every live semaphore). Hoist excess waits
    onto same-engine NOPs placed immediately before the instruction —
    engine queues are sequential, so semantics are preserved."""
    n = 0
    for f in nc.m.functions:
        for bb in f.blocks:
            out = []
            changed = False
            for ins in bb.instructions:
                si = ins.sync_info
                ow = list(si.on_wait) if si is not None else []
                cap = 2 if ins.opcode == "EventSemaphore" else 1
                if len(ow) > cap:
                    sem_waits = [w for w in ow if w.sync_type == "semaphore"]
                    other = [w for w in ow if w.sync_type != "semaphore"]
                    keep = max(cap - len(other), 0)
                    hoist = sem_waits[:len(sem_waits) - keep] if keep else sem_waits
                    kept = sem_waits[len(hoist):] + other
                    assert len(kept) <= cap, (len(kept), cap, ins.opcode)
                    changed = True
                    for w in hoist:
                        n += 1
                        nop = mybir.InstNoOp(
                            name=f"wsplit-{n}",
                            sync_info=mybir.SyncInfo(on_wait=[w], on_update=[]),
                            bass_nofuse=True,
                            engine=ins.engine,
                        )
                        nc.register_instruction(nop, overwrite=True)
                        out.append(nop)
                    si.on_wait = kept
                out.append(ins)
            if changed:
                bb.instructions = out
    return nc


def _build():
    import concourse.bass as bass
    import concourse.mybir as mybir
    import concourse.tile as tile

    f32 = mybir.dt.float32
    bf16 = mybir.dt.bfloat16
    AF = mybir.ActivationFunctionType
    ALU = mybir.AluOpType
    AX = mybir.AxisListType

    from concourse.vector_clock import ScopedClock

    class _TrimTailTC(tile.TileContext):
        # Drop the second kernel-tail all-engine barrier: it only orders
        # the semaphore resets against engine halt, and nothing executes
        # after it. The first barrier (before resets) is kept, so resets
        # still happen on a quiesced machine and re-execution stays safe.
        def _drain_and_barrier(self, tick_clock, wait_clock):
            drain_inst = self.nc.sync.drain()
            wait_clock.add_sem_waits(
                drain_inst.ins, ScopedClock({None: tick_clock.global_clock})
            )
            self.nc.all_engine_barrier()
            assert self.sems is not None
            popped = self.nc._tile_sem_poison_stack.pop()
            assert popped is self._sem_poison
            self.nc.clear_and_free_semaphores(
                list(self.sems.allocated().values())
            )

    nc = bass.Bass()

    d_cbig = nc.dram_tensor("cbig", [128, NCBIG], bf16, kind="ExternalInput")
    d_cf = nc.dram_tensor("cf", [128, NCF], f32, kind="ExternalInput")
    d_dxT = nc.dram_tensor("dxT", [128, KD * N], bf16, kind="ExternalInput")
    d_dyT = nc.dram_tensor("dyT", [128, KD * N], bf16, kind="ExternalInput")
    d_eT = nc.dram_tensor("eT", [128, KN * D], bf16, kind="ExternalInput")
    d_out = nc.dram_tensor("out", [T, D], f32, kind="ExternalOutput")

    with _TrimTailTC(nc) as tc, ExitStack() as ctx:
        work = ctx.enter_context(tc.tile_pool(name="work", bufs=1))
        stats = ctx.enter_context(tc.tile_pool(name="stats", bufs=1))
        p_u = ctx.enter_context(tc.tile_pool(name="p_u", bufs=2, space="PSUM"))
        p_sq = ctx.enter_context(tc.tile_pool(name="p_sq", bufs=3, space="PSUM"))
        p_g = ctx.enter_context(tc.tile_pool(name="p_g", bufs=2, space="PSUM"))
        p_med = ctx.enter_context(tc.tile_pool(name="p_med", bufs=1,
                                               space="PSUM"))

        # ---- ACT table preload: one Ln at boot loads the
        # natural_log_exp_and_others set; every later ACT func is in-set.
        pre_sb = stats.tile([1, 1], f32)
        nc.gpsimd.memset(pre_sb[:], 1.0)
        pre_o = stats.tile([1, 1], f32)
        nc.scalar.activation(pre_o[:], pre_sb[:], AF.Ln)

        # const bias APs (walrus needs non-Copy act biases as APs)
        c_eps = stats.tile([T, 1], f32)
        nc.gpsimd.memset(c_eps[:], LN_EPS)
        c_lneps = stats.tile([T, 1], f32)
        nc.gpsimd.memset(c_lneps[:], math.log(LN_EPS))

        # ---- DMAs: qSP-HWDGE executes these in FIFO order, each striped
        # across the 16 SDMA engines at ~full HBM rate. Few, large pieces.
        cbig_sb = work.tile([128, NCBIG], bf16)
        nc.sync.dma_start(cbig_sb[:], d_cbig[:])
        embT_sb = cbig_sb[:, 0:2 * T]
        emb_sb = cbig_sb[:, 2 * T:2 * T + D]
        ustrict_sb = cbig_sb[:, 2 * T + D:3 * T + D]
        dmaskT_sb = cbig_sb[:, 3 * T + D:4 * T + D]

        # dxT streams immediately after the small const block (it gates u,
        # the head of the dependency chain); cf (trik/ident) is not needed
        # until the C' Exp, so it rides after.
        dxT_sb = work.tile([128, KD * N], bf16)
        for h in range(2):
            nc.sync.dma_start(dxT_sb[:, h * 2048:(h + 1) * 2048],
                              d_dxT[:, h * 2048:(h + 1) * 2048])
        cf_sb = work.tile([128, NCF], f32)
        nc.sync.dma_start(cf_sb[:], d_cf[:])
        trik_sb = cf_sb[:, 0:T]
        ident_sb = cf_sb[:, T:2 * T]
        xdvec_sb = cf_sb[:, 2 * T:2 * T + 1]

        dyT_sb = work.tile([128, KD * N], bf16)
        for h in range(2):
            nc.sync.dma_start(dyT_sb[:, h * 2048:(h + 1) * 2048],
                              d_dyT[:, h * 2048:(h + 1) * 2048])
        eT_sb = work.tile([128, KN * D], bf16)
        for h in range(2):
            nc.sync.dma_start(eT_sb[:, h * 2048:(h + 1) * 2048],
                              d_eT[:, h * 2048:(h + 1) * 2048])

        # ---- u = relu(emb @ Dx.T): t-major; each chunk's evac is split
        # DVE/ACT half-and-half so the tail chunk's latency halves.
        u_sb = work.tile([T, N], bf16)
        su_part = stats.tile([T, 2 * NJ], f32)

        def bridge(lhsT, rhs, cols):
            # Dep-pinned PE filler: reads real SBUF data so the scheduler
            # cannot hoist it ahead of the producer; output is never read.
            # Keeps the tensor engine dense across stalls so the DVFS clock
            # ramp is not reset (full clock needs ~4-5us of dense activity).
            ps = p_g.tile([T, T], f32, tag="g")
            nc.tensor.matmul(ps[:, 0:cols], lhsT, rhs, start=True, stop=True)

        for j in range(NJ):
            ps = p_u.tile([128, 512], f32, tag="pu")
            for k in range(KD):
                nc.tensor.matmul(
                    ps[:],
                    embT_sb[:, k * T:(k + 1) * T],
                    dxT_sb[:, j * 1024 + k * 512: j * 1024 + (k + 1) * 512],
                    start=(k == 0),
                    stop=(k == KD - 1),
                )
            nc.vector.tensor_scalar(
                u_sb[:, j * 512:j * 512 + 256], ps[:, 0:256], 0.0, 0.0,
                op0=ALU.max, op1=ALU.add, accum_out=su_part[:, 2 * j:2 * j + 1],
            )
            nc.scalar.activation(
                u_sb[:, j * 512 + 256:(j + 1) * 512], ps[:, 256:512], AF.Relu,
                accum_out=su_part[:, 2 * j + 1:2 * j + 2],
            )
            if j >= 1:
                bridge(u_sb[:, (j - 1) * 512:(j - 1) * 512 + T],
                       emb_sb[:, 0:T], T)
                bridge(u_sb[:, (j - 1) * 512 + T:(j - 1) * 512 + 2 * T],
                       emb_sb[:, 0:T], T)

        # ---- vn stats (all-ACT, runs while DVE evacs u) ------------------
        junk = work.tile([T, D], bf16)
        msum = stats.tile([T, 1], f32)
        nc.scalar.activation(junk[:], emb_sb, AF.Copy, accum_out=msum[:])
        negm = stats.tile([T, 1], f32)
        nc.scalar.mul(negm[:], msum[:], -1.0 / D)
        ssum = stats.tile([T, 1], f32)
        nc.scalar.activation(junk[:], emb_sb, AF.Square, bias=negm[:],
                             accum_out=ssum[:])
        lv_vn = stats.tile([T, 1], f32)
        nc.scalar.activation(lv_vn[:], ssum[:], AF.Ln, scale=1.0 / D,
                             bias=c_eps[:])
        rstd_vn = stats.tile([T, 1], f32)
        nc.scalar.activation(rstd_vn[:], lv_vn[:], AF.Exp, scale=-0.5)
        nmr_vn = stats.tile([T, 1], f32)
        nc.scalar.mul(nmr_vn[:], negm[:], rstd_vn[:])

        # ---- C' matrix: q -> strict cumsum -> one Exp with bias ----------
        su = stats.tile([T, 1], f32)
        nc.vector.tensor_reduce(su[:], su_part[:], axis=AX.X, op=ALU.add)
        q_sb = stats.tile([T, 1], bf16)
        nc.scalar.activation(q_sb[:], su[:], AF.Ln, scale=C2, bias=xdvec_sb[:])
        colsc = p_sq.tile([T, T], f32, tag="sq")            # Q_s - q_s column
        nc.tensor.matmul(colsc[:, 0:1], ustrict_sb[:], q_sb[:], start=True,
                         stop=True)
        colsc_sb = stats.tile([T, 1], f32)
        nc.vector.tensor_copy(colsc_sb[:], colsc[:, 0:1])
        ct_sb = work.tile([T, T], bf16)                     # C'[s,t]
        nc.scalar.activation(ct_sb[:], trik_sb[:], AF.Exp, bias=colsc_sb[:])

        # ---- scale-absorption factors (off critical path) ----------------
        qq = stats.tile([T, 1], f32)                        # Q_s inclusive
        nc.vector.tensor_tensor(qq[:], colsc[:, 0:1], q_sb[:], op=ALU.add)
        expQ = stats.tile([T, 1], f32)                      # e^{-Q_s}
        nc.scalar.activation(expQ[:], qq[:], AF.Exp, scale=-1.0)
        epsQ = stats.tile([T, 1], f32)                      # eps * e^{2 Q_t}
        nc.scalar.activation(epsQ[:], qq[:], AF.Exp, scale=2.0,
                             bias=c_lneps[:])
        rstd2 = stats.tile([T, 1], f32)
        nc.scalar.mul(rstd2[:], rstd_vn[:], expQ[:])
        nmr2 = stats.tile([T, 1], f32)
        nc.scalar.mul(nmr2[:], nmr_vn[:], expQ[:])
        vn_sb = work.tile([T, D], bf16)                     # e^{-Q} * LN(emb)
        nc.scalar.activation(vn_sb[:], emb_sb, AF.Identity,
                             scale=rstd2[:], bias=nmr2[:])

        # ---- X^T directly (n-major): XT_c = u_c^T(stationary) @ C'^T;
        # Gram super-chunks G_q = sum_c XT_c XT_c^T; a* += (G_q.dmask) @ vn.
        xt_sb = work.tile([128, N], bf16)
        aps = p_med.tile([T, D], f32, tag="med")
        gq_ps = {}

        def xt_block(q):
            for cc in range(4):
                c = 4 * q + cc
                tp = p_sq.tile([128, T], f32, tag="sq")
                nc.tensor.matmul(tp[:], u_sb[:, c * T:(c + 1) * T], ct_sb[:],
                                 start=True, stop=True)
                # GPSIMD cannot read PSUM; alternate DVE / ACT for evacs.
                if c % 2 == 0:
                    nc.vector.tensor_copy(xt_sb[:, c * T:(c + 1) * T], tp[:])
                else:
                    nc.scalar.copy(xt_sb[:, c * T:(c + 1) * T], tp[:])

        def g_block(q):
            g = p_g.tile([T, T], f32, tag="g")
            gq_ps[q] = g
            for cc in range(4):
                c = 4 * q + cc
                nc.tensor.matmul(g[:], xt_sb[:, c * T:(c + 1) * T],
                                 xt_sb[:, c * T:(c + 1) * T],
                                 start=(cc == 0), stop=(cc == 3))

        def astar_block(q):
            g = gq_ps.pop(q)
            wt = work.tile([T, T], bf16, tag=f"wt{q % 2}")
            nc.vector.tensor_tensor(wt[:], g[:], dmaskT_sb[:], op=ALU.mult)
            nc.tensor.matmul(aps[:], wt[:], vn_sb[:], start=(q == 0),
                             stop=(q == 3))

        xt_block(0)
        xt_block(1)
        g_block(0)
        xt_block(2)
        g_block(1)
        astar_block(0)
        xt_block(3)
        g_block(2)
        astar_block(1)
        g_block(3)
        astar_block(2)
        astar_block(3)

        # ---- lna = LN(a*) (f32 for the PE transposes), lnaT bf16.
        # Stats first; PE bridges pinned on stat6 keep the clock dense
        # through the ~2us LN latency; the normalize runs split in halves
        # so transpose k0 starts while half 1 normalizes.
        lna_sb = work.tile([T, D], f32)
        la_s6 = stats.tile([T, 6], f32)
        nc.vector.bn_stats(la_s6[:], aps[:])
        la_mv = stats.tile([T, 2], f32)
        nc.vector.bn_aggr(la_mv[:], la_s6[:])
        la_mvb = stats.tile([T, 2], bf16)
        nc.vector.tensor_copy(la_mvb[:], la_mv[:])
        for i in range(10):
            bridge(xt_sb[:, (i % 4) * T:(i % 4 + 1) * T], la_mvb[:], 2)
        la_lv = stats.tile([T, 1], f32)
        nc.scalar.activation(la_lv[:], la_mv[:, 1:2], AF.Ln, bias=epsQ[:])
        la_rs = stats.tile([T, 1], f32)
        nc.scalar.activation(la_rs[:], la_lv[:], AF.Exp, scale=-0.5)
        lnaT_sb = work.tile([128, KD * T], bf16)
        for k in range(KD):
            nc.vector.tensor_scalar(
                lna_sb[:, k * T:(k + 1) * T], aps[:, k * T:(k + 1) * T],
                la_mv[:, 0:1], la_rs[:], op0=ALU.subtract, op1=ALU.mult)
            tp = p_sq.tile([T, T], f32, tag="sq")
            nc.tensor.transpose(tp[:], lna_sb[:, k * T:(k + 1) * T], ident_sb)
            nc.scalar.copy(lnaT_sb[:, k * T:(k + 1) * T], tp[:])

        # ---- Ycore^T directly (n-major): yc_c = Dy_c(stationary) @ lnaT;
        # yT_c = relu(yc_c) * xT_c on DVE; vraw += yT_c^T @ eT_c. One chunk
        # of PE lookahead hides each chunk's DVE evac.
        yt_sb = work.tile([128, N], bf16)
        vps = p_med.tile([T, D], f32, tag="med")
        yc_ps = {}

        def yc_block(c):
            j = c // 4
            r = c % 4
            ps = p_sq.tile([128, T], f32, tag="sq")
            yc_ps[c] = ps
            for k in range(KD):
                nc.tensor.matmul(
                    ps[:],
                    dyT_sb[:, j * 1024 + k * 512 + r * T:
                           j * 1024 + k * 512 + (r + 1) * T],
                    lnaT_sb[:, k * T:(k + 1) * T],
                    start=(k == 0),
                    stop=(k == KD - 1),
                )

        def yt_evac(c):
            ps = yc_ps.pop(c)
            nc.vector.scalar_tensor_tensor(
                yt_sb[:, c * T:(c + 1) * T], ps[:], 0.0,
                xt_sb[:, c * T:(c + 1) * T], op0=ALU.max, op1=ALU.mult,
            )

        def vraw_block(c):
            nc.tensor.matmul(vps[:], yt_sb[:, c * T:(c + 1) * T],
                             eT_sb[:, c * D:(c + 1) * D],
                             start=(c == 0), stop=(c == KN - 1))

        yc_block(0)
        yt_evac(0)
        yc_block(1)
        yt_evac(1)
        for c in range(2, KN):
            yc_block(c)
            yt_evac(c)
            vraw_block(c - 2)
        vraw_block(KN - 2)
        vraw_block(KN - 1)

        # ---- v* = LN(vraw) with the same per-row eps, f32 out; the
        # normalize and writeback run split in halves so the first DMA
        # issues while the second half normalizes.
        vstar_sb = work.tile([T, D], f32)
        vs_s6 = stats.tile([T, 6], f32)
        nc.vector.bn_stats(vs_s6[:], vps[:])
        vs_mv = stats.tile([T, 2], f32)
        nc.vector.bn_aggr(vs_mv[:], vs_s6[:])
        vs_lv = stats.tile([T, 1], f32)
        nc.scalar.activation(vs_lv[:], vs_mv[:, 1:2], AF.Ln, bias=epsQ[:])
        vs_rs = stats.tile([T, 1], f32)
        nc.scalar.activation(vs_rs[:], vs_lv[:], AF.Exp, scale=-0.5)
        for k in range(KD):
            nc.vector.tensor_scalar(
                vstar_sb[:, k * T:(k + 1) * T], vps[:, k * T:(k + 1) * T],
                vs_mv[:, 0:1], vs_rs[:], op0=ALU.subtract, op1=ALU.mult)
            nc.sync.dma_start(d_out[:, k * T:(k + 1) * T],
                              vstar_sb[:, k * T:(k + 1) * T])

    return _split_multiwait(nc, mybir)


def _numpy_fallback(embeddings, E, Dx, Dy, x_state, rho_state):
    # General-path reference (only used if initial states are nonzero).
    def ln(x):
        m = x.mean(-1, keepdims=True)
        v = ((x - m) ** 2).mean(-1, keepdims=True)
        return (x - m) / np.sqrt(v + LN_EPS)

    x_s = x_state.astype(np.float32).copy()
    rho = rho_state.astype(np.float32).copy()
    outs = np.zeros((B, T, D), dtype=np.float32)
    for t in range(T):
        v_prev = embeddings[:, t, :]
        x_upd = np.maximum(v_prev @ Dx.T, 0.0)
        x_t = XD * x_s + x_upd
        x_t = x_t / np.maximum(np.abs(x_t).sum(-1, keepdims=True), L1_EPS)
        a_star = np.einsum("bdn,bn->bd", rho, x_t)
        y_core = ln(a_star) @ Dy.T
        y_t = np.maximum(y_core, 0.0) * np.maximum(x_t, 0.0)
        outs[:, t, :] = ln(y_t @ E.T)
        vn = ln(v_prev)
        rho = UD * rho + np.einsum("bd,bn->bdn", vn, x_t)
        x_s = x_t
    return outs


def kernel(embeddings, E, Dx, Dy, x_state, rho_state):
    embeddings = np.ascontiguousarray(embeddings, dtype=np.float32)
    E = np.ascontiguousarray(E, dtype=np.float32)
    Dx = np.ascontiguousarray(Dx, dtype=np.float32)
    Dy = np.ascontiguousarray(Dy, dtype=np.float32)

    if np.any(x_state) or np.any(rho_state):
        return _numpy_fallback(embeddings, E, Dx, Dy,
                               np.asarray(x_state, np.float32),
                               np.asarray(rho_state, np.float32))

    from concourse.bass_utils import run_bass_kernel_spmd

    if "nc" not in _cache:
        _cache["nc"] = _build()
    nc = _cache["nc"]

    cf = _consts_f()
    cb = _consts_b()
    dxT = _pack_jk(Dx.T.reshape(KD, 128, N)).astype(BF16)
    dyT = _pack_jk(Dy.T.reshape(KD, 128, N)).astype(BF16)
    eT = np.ascontiguousarray(
        E.T.reshape(KN, 128, D).transpose(1, 0, 2).reshape(128, KN * D)
    ).astype(BF16)

    in_maps = []
    for b in range(B):
        emb_b = embeddings[b]
        embT_b = np.ascontiguousarray(
            emb_b.T.reshape(KD, 128, T).transpose(1, 0, 2).reshape(128, KD * T)
        ).astype(BF16)
        cbig = np.ascontiguousarray(np.concatenate(
            [embT_b, emb_b.astype(BF16), cb], axis=1))
        in_maps.append({
            "cbig": cbig,
            "cf": cf,
            "dxT": dxT,
            "dyT": dyT,
            "eT": eT,
        })

    res = run_bass_kernel_spmd(nc, in_maps, list(range(B)))
    _cache["last_results"] = res
    return np.stack([res.results[i]["out"] for i in range(B)])
